# revision 1
# baseline (speedup 1.0000x reference)
"""AttentionBlock Trainium2 kernel: GroupNorm -> QKV -> MHA -> proj -> residual.

Data-parallel over batch B=8 across 8 NeuronCores (one batch image per core).
All matmuls run in bf16 on the TensorEngine (fp32 accumulation in PSUM);
GroupNorm statistics and the residual path stay in fp32.

Per-core layouts (C=512 channels, HW=1024 tokens, 8 heads, hd=64):
  x, xn        [C, HW]   channels on partitions (4 chunks of 128)
  q, k         [C_qk, HW] = qk_sb[128, 8 oc, 1024]; head pair hp lives in
               oc=hp (q) / oc=4+hp (k), heads at partition 0:64 / 64:128
  vT           [HW, C_v] = vt[128, 8 hwc, 8 head, 65] with a ones column
               (65th) so the attention-value matmul also produces the
               softmax denominator.
  scoresT      [k, q] per (head, kchunk): PSUM [128, 1024]
  E=exp(s*sc)  SBUF bf16 per pair: [128, 2 head, 8 kc, 1024]
  att          [C, HW] bf16 (pair hp -> chunk hp)
  out          [C, HW] fp32 = proj(att) + proj_b + x
"""

import sys

if "/opt/trn_rl_repo" not in sys.path:
    sys.path.insert(0, "/opt/trn_rl_repo")

import numpy as np
import ml_dtypes

import concourse.bass as bass
import concourse.tile as tile
from concourse import mybir, bacc
from concourse.bass_utils import run_bass_kernel_spmd

AF = mybir.ActivationFunctionType
ALU = mybir.AluOpType
F32 = mybir.dt.float32
BF16 = mybir.dt.bfloat16

C = 512
HW = 1024
NHEADS = 8
HD = 64
NGROUPS = 32
GSIZE = 16  # channels per group
EPS = 1e-5
SCALE = HD ** -0.5
CC = 4   # channel chunks of 128
OCQK = 8  # q+k output chunks of 128
QC = 2   # 512-wide moving slices per 1024


def _build():
    nc = bacc.Bacc("TRN2", target_bir_lowering=False, debug=False, num_devices=8)

    x_d = nc.dram_tensor("x", [C, HW], F32, kind="ExternalInput")
    qw_d = nc.dram_tensor("qw", [C, 3 * C], BF16, kind="ExternalInput")
    pw_d = nc.dram_tensor("pw", [C, C], BF16, kind="ExternalInput")
    qkb_d = nc.dram_tensor("qkb", [128, 8], F32, kind="ExternalInput")
    vbb_d = nc.dram_tensor("vbb", [128, C], F32, kind="ExternalInput")
    pb_d = nc.dram_tensor("pb", [128, 4], F32, kind="ExternalInput")
    gnw_d = nc.dram_tensor("gnw", [128, 4], F32, kind="ExternalInput")
    gnb_d = nc.dram_tensor("gnb", [128, 4], F32, kind="ExternalInput")
    ind_d = nc.dram_tensor("ind", [128, 8], F32, kind="ExternalInput")
    indt_d = nc.dram_tensor("indt", [8, 128], F32, kind="ExternalInput")
    out_d = nc.dram_tensor("out", [C, HW], F32, kind="ExternalOutput")

    with tile.TileContext(nc) as tc:
        with (
            tc.tile_pool(name="consts", bufs=1) as consts,
            tc.tile_pool(name="epool", bufs=2) as epool,
            tc.tile_pool(name="small", bufs=4) as small,
            tc.tile_pool(name="outp", bufs=3) as outp,
            tc.tile_pool(name="drp", bufs=4, space="DRAM") as drp,
            tc.tile_pool(name="ps_s", bufs=2, space="PSUM") as ps_s,
            tc.tile_pool(name="ps_av", bufs=2, space="PSUM") as ps_av,
        ):
            # ---- persistent SBUF tiles + input DMAs ----
            # x split per chunk so GroupNorm stats start after the first 512KB
            x_sb = consts.tile([128, CC, HW], F32, tag="x")
            x_r = x_d.ap().rearrange("(cc p) hw -> p cc hw", p=128)
            for cc in range(CC):
                nc.sync.dma_start(out=x_sb[:, cc, :], in_=x_r[:, cc, :])
            qw_sb = consts.tile([128, CC, 3 * C], BF16, tag="qw")
            nc.sync.dma_start(out=qw_sb, in_=qw_d.ap().rearrange("(cc p) o -> p cc o", p=128))
            pw_sb = consts.tile([128, CC, C], BF16, tag="pw")
            nc.sync.dma_start(out=pw_sb, in_=pw_d.ap().rearrange("(cc p) o -> p cc o", p=128))
            qkb = consts.tile([128, 8], F32, tag="qkb")
            nc.sync.dma_start(out=qkb, in_=qkb_d.ap())
            vbb = consts.tile([128, C], F32, tag="vbb")
            nc.sync.dma_start(out=vbb, in_=vbb_d.ap())
            pb = consts.tile([128, 4], F32, tag="pb")
            nc.sync.dma_start(out=pb, in_=pb_d.ap())
            gnw = consts.tile([128, 4], F32, tag="gnw")
            nc.sync.dma_start(out=gnw, in_=gnw_d.ap())
            gnb = consts.tile([128, 4], F32, tag="gnb")
            nc.sync.dma_start(out=gnb, in_=gnb_d.ap())
            ind = consts.tile([128, 8], F32, tag="ind")
            nc.sync.dma_start(out=ind, in_=ind_d.ap())
            indt = consts.tile([8, 128], F32, tag="indt")
            nc.sync.dma_start(out=indt, in_=indt_d.ap())

            xn_sb = consts.tile([128, CC, HW], BF16, tag="xn")
            qk_sb = consts.tile([128, OCQK, HW], BF16, tag="qk")
            vt_sb = consts.tile([128, 8, NHEADS, HD + 1], BF16, tag="vt")
            att_t = [consts.tile([128, HW], BF16, tag=f"att{i}", name=f"att{i}") for i in range(CC)]

            # ones column of vT (softmax denominator trick)
            nc.vector.memset(vt_sb[:, :, :, HD:HD + 1], 1.0)

            # ---- GroupNorm (batched across the 4 channel chunks) ----
            # per-channel stats: one bn_stats over [128, 8, 512], one bn_aggr per chunk
            st = small.tile([128, CC, 2, 6], F32, tag="gn_st")
            for cc in range(CC):
                nc.vector.bn_stats(out=st[:, cc, 0, :], in_=x_sb[:, cc, 0:512])
                nc.vector.bn_stats(out=st[:, cc, 1, :], in_=x_sb[:, cc, 512:1024])
            mv = small.tile([128, CC, 2], F32, tag="gn_mv")
            for cc in range(CC):
                nc.vector.bn_aggr(out=mv[:, cc, :], in_=st[:, cc, :, :])
            # mv col1 <- E[x^2]_c = var_c + mean_c^2 (in place)
            scr = small.tile([128, CC, 1], F32, tag="gn_scr")
            nc.vector.tensor_mul(out=scr, in0=mv[:, :, 0:1], in1=mv[:, :, 0:1])
            nc.vector.tensor_add(out=mv[:, :, 1:2], in0=mv[:, :, 1:2], in1=scr)
            # group means over 16-channel blocks (ind carries the 1/16): [8, 4cc*2]
            pg = ps_s.tile([8, CC, 2], F32, tag="ps_s")
            nc.tensor.matmul(out=pg, lhsT=ind[:, :], rhs=mv.rearrange("p cc s -> p (cc s)"), start=True, stop=True)
            sg = small.tile([8, CC, 2], F32, tag="gn_sg")
            nc.vector.tensor_copy(out=sg, in_=pg)
            # vpe = E[x^2]_g - mean_g^2 + eps
            vg = small.tile([8, CC, 3], F32, tag="gn_vg")
            nc.vector.scalar_tensor_tensor(out=vg[:, :, 0:1], in0=sg[:, :, 0:1], scalar=-1.0, in1=sg[:, :, 0:1], op0=ALU.mult, op1=ALU.mult)
            nc.vector.scalar_tensor_tensor(out=vg[:, :, 1:2], in0=sg[:, :, 1:2], scalar=EPS, in1=vg[:, :, 0:1], op0=ALU.add, op1=ALU.add)
            # rstd = 1/sqrt(vpe) with one Newton polish (ACT sqrt is low-precision)
            rs = small.tile([8, CC, 3], F32, tag="gn_rs")
            nc.scalar.activation(out=rs[:, :, 0:1].rearrange("g cc one -> g (cc one)"), in_=vg[:, :, 1:2].rearrange("g cc one -> g (cc one)"), func=AF.Sqrt, bias=0.0, scale=1.0)
            nc.vector.reciprocal(out=rs[:, :, 1:2], in_=rs[:, :, 0:1])
            nc.vector.scalar_tensor_tensor(out=rs[:, :, 2:3], in0=rs[:, :, 1:2], scalar=1.0, in1=rs[:, :, 1:2], op0=ALU.mult, op1=ALU.mult)
            nc.vector.scalar_tensor_tensor(out=rs[:, :, 2:3], in0=rs[:, :, 2:3], scalar=-0.5, in1=vg[:, :, 1:2], op0=ALU.mult, op1=ALU.mult)
            nc.vector.scalar_tensor_tensor(out=sg[:, :, 1:2], in0=rs[:, :, 2:3], scalar=1.5, in1=rs[:, :, 1:2], op0=ALU.add, op1=ALU.mult)
            # broadcast [mean_g, rstd_g] to channels: [128, 4cc*2] = indt.T @ sg
            pbc = ps_av.tile([128, CC, 2], F32, tag="ps_av")
            nc.tensor.matmul(out=pbc, lhsT=indt[:, :], rhs=sg.rearrange("g cc s -> g (cc s)"), start=True, stop=True)
            # A = rstd_bc * gnw ; B = gnb - mean_bc * A (all chunks)
            ab = small.tile([128, CC, 2], F32, tag="gn_ab")
            nc.vector.tensor_mul(out=ab[:, :, 0:1], in0=pbc[:, :, 1:2], in1=gnw.rearrange("p (cc one) -> p cc one", one=1))
            nc.vector.scalar_tensor_tensor(out=ab[:, :, 1:2], in0=pbc[:, :, 0:1], scalar=-1.0, in1=ab[:, :, 0:1], op0=ALU.mult, op1=ALU.mult)
            nc.vector.tensor_add(out=ab[:, :, 1:2], in0=gnb.rearrange("p (cc one) -> p cc one", one=1), in1=ab[:, :, 1:2])
            for cc in range(CC):
                nc.vector.tensor_scalar(out=xn_sb[:, cc, :], in0=x_sb[:, cc, :], scalar1=ab[:, cc, 0:1], scalar2=ab[:, cc, 1:2], op0=ALU.mult, op1=ALU.add)

            # ---- V^T = xn^T @ v_w^T  ([hw, o] layout, interleaved per head) ----
            for hwc in range(8):
                pv = ps_av.tile([128, 512], F32, tag="ps_av", name=f"pv{hwc}")
                for cc in range(CC):
                    nc.tensor.matmul(
                        out=pv,
                        lhsT=xn_sb[:, cc, hwc * 128:(hwc + 1) * 128],
                        rhs=qw_sb[:, cc, 2 * C:3 * C],
                        start=(cc == 0), stop=(cc == CC - 1),
                    )
                nc.vector.tensor_add(
                    out=vt_sb[:, hwc, :, 0:HD],
                    in0=pv[:].rearrange("p (h d) -> p h d", d=HD),
                    in1=vbb[:].rearrange("p (h d) -> p h d", d=HD),
                )

            # ---- attention (software-pipelined across head pairs) ----
            # Per pair hp: scores+exp stream per kchunk; head a=1's AV streams
            # inside the pair (one ps_av slot); head a=0's AV runs as a batch
            # early in the NEXT pair (second ps_av slot). QK psums for pair
            # hp+1 are produced mid-pair on the ps_av pool so the ps_s pool
            # stays dedicated to the scores->exp stream.
            def make_qk(oc):
                pq = ps_av.tile([128, HW], F32, tag="ps_av", name=f"pq{oc}")
                for cc in range(CC):
                    for q2 in range(QC):
                        nc.tensor.matmul(
                            out=pq[:, q2 * 512:(q2 + 1) * 512],
                            lhsT=qw_sb[:, cc, oc * 128:(oc + 1) * 128],
                            rhs=xn_sb[:, cc, q2 * 512:(q2 + 1) * 512],
                            start=(cc == 0), stop=(cc == CC - 1),
                        )
                nc.vector.tensor_scalar_add(out=qk_sb[:, oc, :], in0=pq[:], scalar1=qkb[:, oc:oc + 1])

            def make_qk_first(oc):
                # lead-in variant on ps_s (ps_av is busy with V^T production)
                pq = ps_s.tile([128, HW], F32, tag="ps_s", name=f"pq{oc}")
                for cc in range(CC):
                    for q2 in range(QC):
                        nc.tensor.matmul(
                            out=pq[:, q2 * 512:(q2 + 1) * 512],
                            lhsT=qw_sb[:, cc, oc * 128:(oc + 1) * 128],
                            rhs=xn_sb[:, cc, q2 * 512:(q2 + 1) * 512],
                            start=(cc == 0), stop=(cc == CC - 1),
                        )
                nc.vector.tensor_scalar_add(out=qk_sb[:, oc, :], in0=pq[:], scalar1=qkb[:, oc:oc + 1])

            make_qk_first(0)
            make_qk_first(4)

            def normalize_head(hp, a, av_tile):
                # copy AV block PSUM->SBUF (releases the PSUM slot), then
                # broadcast the denominator row and divide
                avs = small.tile([65, HW], F32, tag="avs", name=f"avs{hp}_{a}")
                nc.vector.tensor_copy(out=avs, in_=av_tile[:, :])
                # reciprocal of the denominator row, then broadcast to 64
                # partitions via a DRAM bounce (DMA handles partition fan-out)
                nc.vector.reciprocal(out=avs[64:65, :], in_=avs[64:65, :])
                dscr = drp.tile([HW], F32, tag="dscr", name=f"dscr{hp}_{a}")
                nc.sync.dma_start(out=dscr, in_=avs[64:65, :])
                sbc = small.tile([64, HW], F32, tag="sbc", name=f"sbc{hp}_{a}")
                dap = dscr
                bcast = bass.AP(tensor=dap.tensor, offset=dap.offset, ap=[[0, 64]] + list(dap.ap))
                nc.sync.dma_start(out=sbc, in_=bcast)
                if a == 0:
                    nc.vector.tensor_mul(out=att_t[hp][0:64, :], in0=avs[0:64, :], in1=sbc)
                else:
                    sc = small.tile([64, HW], BF16, tag="att_sc", name=f"attsc{hp}_{a}")
                    nc.vector.tensor_mul(out=sc, in0=avs[0:64, :], in1=sbc)
                    nc.sync.dma_start(out=att_t[hp][64:128, :], in_=sc)

            def av_batch(hp, a, E_tile):
                av0 = ps_av.tile([65, HW], F32, tag="ps_av", name=f"av{a}_{hp}")
                for kc in range(8):
                    for q2 in range(QC):
                        nc.tensor.matmul(
                            out=av0[:, q2 * 512:(q2 + 1) * 512],
                            lhsT=vt_sb[:, kc, 2 * hp + a, :],
                            rhs=E_tile[:, a, kc, q2 * 512:(q2 + 1) * 512],
                            start=(kc == 0), stop=(kc == 7),
                        )
                normalize_head(hp, a, av0)

            E_prev = None
            for hp in range(4):
                E = epool.tile([128, 2, 8, HW], BF16, tag="E", name=f"E{hp}")
                av1 = ps_av.tile([65, HW], F32, tag="ps_av", name=f"av1_{hp}")
                for kc in range(8):
                    for a in (1, 0):
                        ps = ps_s.tile([128, HW], F32, tag="ps_s", name=f"ps{hp}_{kc}_{a}")
                        for q2 in range(QC):
                            nc.tensor.matmul(
                                out=ps[:, q2 * 512:(q2 + 1) * 512],
                                lhsT=qk_sb[a * 64:(a + 1) * 64, 4 + hp, kc * 128:(kc + 1) * 128],
                                rhs=qk_sb[a * 64:(a + 1) * 64, hp, q2 * 512:(q2 + 1) * 512],
                                start=True, stop=True,
                            )
                        nc.scalar.activation(out=E[:, a, kc, :], in_=ps[:], func=AF.Exp, scale=SCALE)
                    # head a=1 AV streams within the pair
                    for q2 in range(QC):
                        nc.tensor.matmul(
                            out=av1[:, q2 * 512:(q2 + 1) * 512],
                            lhsT=vt_sb[:, kc, 2 * hp + 1, :],
                            rhs=E[:, 1, kc, q2 * 512:(q2 + 1) * 512],
                            start=(kc == 0), stop=(kc == 7),
                        )
                    if kc == 1 and E_prev is not None:
                        av_batch(hp - 1, 0, E_prev)
                    if kc == 4 and hp + 1 < 4:
                        make_qk(hp + 1)
                    if kc == 6 and hp + 1 < 4:
                        make_qk(4 + hp + 1)
                normalize_head(hp, 1, av1)
                E_prev = E

            # ---- tail: last pair's a=0 AV batch + proj ----
            av_batch(3, 0, E_prev)

            def proj_mm(py, oc, cc):
                for q2 in range(QC):
                    nc.tensor.matmul(
                        out=py[:, q2 * 512:(q2 + 1) * 512],
                        lhsT=pw_sb[:, cc, oc * 128:(oc + 1) * 128],
                        rhs=att_t[cc][:, q2 * 512:(q2 + 1) * 512],
                        start=(cc == 0), stop=(cc == CC - 1),
                    )

            def proj_epilogue(py, oc):
                ot = outp.tile([128, HW], F32, tag="ot", name=f"ot{oc}")
                nc.vector.scalar_tensor_tensor(out=ot, in0=py[:], scalar=pb[:, oc:oc + 1], in1=x_sb[:, oc, :], op0=ALU.add, op1=ALU.add)
                nc.sync.dma_start(out=out_d.ap()[oc * 128:(oc + 1) * 128, :], in_=ot)

            # phase A: att3-independent contraction overlaps pair 3's tail
            py01 = []
            for oc in range(2):
                py = ps_s.tile([128, HW], F32, tag="ps_s", name=f"py{oc}")
                py01.append(py)
                for cc in range(CC - 1):
                    proj_mm(py, oc, cc)
            for oc in range(2):
                proj_mm(py01[oc], oc, CC - 1)
                proj_epilogue(py01[oc], oc)
            for oc in range(2, CC):
                py = ps_s.tile([128, HW], F32, tag="ps_s", name=f"py{oc}")
                for cc in range(CC):
                    proj_mm(py, oc, cc)
                proj_epilogue(py, oc)

    nc.compile()
    return nc


_NC_CACHE = None


def _get_nc():
    global _NC_CACHE
    if _NC_CACHE is None:
        _NC_CACHE = _build()
    return _NC_CACHE


def _prep_in_maps(inputs):
    x = np.asarray(inputs["x"], np.float32)
    gn_w = np.asarray(inputs["gn_w"], np.float32)
    gn_b = np.asarray(inputs["gn_b"], np.float32)
    qkv_w = np.asarray(inputs["qkv_w"], np.float32)
    qkv_b = np.asarray(inputs["qkv_b"], np.float32)
    proj_w = np.asarray(inputs["proj_w"], np.float32)
    proj_b = np.asarray(inputs["proj_b"], np.float32)

    B = x.shape[0]
    xr = x.reshape(B, C, HW)
    qwT = np.ascontiguousarray(qkv_w.T).astype(ml_dtypes.bfloat16)
    pwT = np.ascontiguousarray(proj_w.T).astype(ml_dtypes.bfloat16)
    qkb = np.ascontiguousarray(qkv_b[: 2 * C].reshape(8, 128).T)
    vbb = np.ascontiguousarray(np.broadcast_to(qkv_b[2 * C:], (128, C)))
    pb = np.ascontiguousarray(proj_b.reshape(4, 128).T)
    gnw = np.ascontiguousarray(gn_w.reshape(4, 128).T)
    gnb = np.ascontiguousarray(gn_b.reshape(4, 128).T)
    indm = np.zeros((128, 8), np.float32)
    indm[np.arange(128), np.arange(128) // GSIZE] = 1.0 / GSIZE
    ind01 = np.zeros((128, 8), np.float32)
    ind01[np.arange(128), np.arange(128) // GSIZE] = 1.0
    indt = np.ascontiguousarray(ind01.T)
    shared = dict(qw=qwT, pw=pwT, qkb=qkb, vbb=vbb, pb=pb, gnw=gnw, gnb=gnb, ind=indm, indt=indt)
    return [dict(x=np.ascontiguousarray(xr[b]), **shared) for b in range(B)]


def kernel(**inputs):
    nc = _get_nc()
    in_maps = _prep_in_maps(inputs)
    res = run_bass_kernel_spmd(nc, in_maps, core_ids=list(range(8)))
    out = np.stack([r["out"] for r in res.results])
    return out.reshape(8, C, 32, 32).astype(np.float32)


def run_profiled(inputs):
    """kernel() + NTFF profiling; returns (output, exec_time_ns, trace_path)."""
    import types

    import antenv

    if "antenv.axon_hooks" not in sys.modules:
        hooks_mod = types.ModuleType("antenv.axon_hooks")
        _hook = [None]
        hooks_mod.set_axon_ntff_profile_hook = lambda h: _hook.__setitem__(0, h)
        hooks_mod.get_axon_ntff_profile_hook = lambda: _hook[0]
        sys.modules["antenv.axon_hooks"] = hooks_mod
        antenv.axon_hooks = hooks_mod
        from trn_agent_boot.trn_boot import _ntff_profile_via_ctypes

        hooks_mod.set_axon_ntff_profile_hook(_ntff_profile_via_ctypes("/opt/axon/libaxon_pjrt.so"))

    nc = _get_nc()
    in_maps = _prep_in_maps(inputs)
    res = run_bass_kernel_spmd(nc, in_maps, core_ids=list(range(8)), trace=True)
    out = np.stack([r["out"] for r in res.results]).reshape(8, C, 32, 32).astype(np.float32)
    trace = res.instructions_and_trace[1] if res.instructions_and_trace else None
    return out, res.exec_time_ns, trace

